# revision 17
# baseline (speedup 1.0000x reference)
"""ExpertLinear (dense MoE blend) Trainium2 kernel — expert-parallel.

y[b,o] = sum_k ew[b,k] * (x[b,:] @ W[k,o,:] + bias[k,o])

Sharding: expert-parallel across 8 cores (core k owns expert k). Each core
computes the full partial z_k = ew[:,k] * (x @ W[k].T + bias[k]) for ALL
512 rows; the host unshard step is a pure sum of the 8 partials. Per-core
HBM traffic is ~4.3 MB (W_k 2 MB bf16 + xT 1 MB bf16 + z_k 1 MB bf16 out)
vs 18.6 MB for the data-parallel layout, because each expert's weights are
read exactly once chip-wide.

Device flow per core (all operands packed in ONE bf16 dram tensor):
  - PE: per o-block group g (8 of them), 8 matmuls (lhsT = W chunk
    [128i,128o] stationary, rhs = xT chunk [128i,512b] moving) accumulate
    zT_k group [128o,512b] into PSUM bank g; all 8 banks live at once,
    each evicted as soon as its group closes so only the last eviction is
    exposed.
  - DVE evict: z = (ps + bias_col) * ew_bcast, downcast bf16. The ew
    blend and bias stay on device; the host only sums partials.
  - Walrus accepts ONE sync wait per instruction and tile emits a sem
    wait for EVERY data dep (even same-engine), so: absorber matmuls
    cover each in-DMA's sem on the PE queue (later matmuls on the same
    sem are coverage-deduped), a tiny DVE copy covers the ew/bias load,
    per-group tmp buffers kill the evict WAR chain, and the DMA count is
    capped at 8 (5 in + 3 out) so no DMAHW sem lane is ever reused (a
    reused lane adds a second wait to that DMA).
  - All DMAs ride the single qSPDynamicHW FIFO ring: the 5 in-DMAs
    stream back-to-back at full bandwidth in issue order, sized so
    compute can start after ~0.8 MB; out-DMAs queue behind them and
    never steal in-stream bandwidth. ~20 warmup matmuls run during the
    initial DMA fill to lift the PE HAM clock gate (1.2 -> 2.4 GHz).
"""

import numpy as np

B, E, IN, OUT = 512, 8, 1024, 1024
NCORES = 8
P = 128
NI = IN // P    # 8 i-chunks (contraction)
NG = OUT // P   # 8 o-block groups
NW = 26         # warmup matmuls (bridge PE from preamble exit to first W arrival)

# input column layout (bf16, [128, IN_COLS])
XTA_C = 0                     # xT chunks 0-3: [128, 4*512]
WG0_C = XTA_C + 4 * B         # W group 0: [128, 8*128]
XTB_C = WG0_C + NI * P        # xT chunks 4-7: [128, 4*512]
EWB_C = XTB_C + 4 * B         # ew column broadcast: [128, 512]
BIAS_C = EWB_C + B            # biasT: [128, 8]
WGR_C = BIAS_C + NG           # W groups 1-7: [128, 7*8*128]
IN_COLS = WGR_C + (NG - 1) * NI * P

# in-DMA boundaries, cut so each segment lands just before PE needs it:
# [xTa|Wg0], [xTb|ewb|biasT], [Wg1], [Wg2|Wg3], [Wg4-7]
D0_E = XTB_C
D1_E = WGR_C
D2_E = WGR_C + NI * P
D3_E = WGR_C + 3 * NI * P


def _wcol(g, i):
    base = WG0_C if g == 0 else WGR_C + (g - 1) * NI * P
    return base + i * P


def _xcol(i):
    return (XTA_C if i < 4 else XTB_C - 4 * B) + i * B


_compiled = None


def _patch_drain_split():
    """The walrus build in this container rejects any instruction carrying
    more than one sync wait, including the kernel-tail Drain that
    TileContext emits with one wait per active semaphore. Split it into a
    sequence of single-wait drains (sequencer-FIFO keeps them ordered;
    the set of waits is identical)."""
    import concourse.tile as tile_mod

    if getattr(tile_mod.TileContext, "_drain_split_patched", False):
        return
    from concourse.tile_sem_assignment import N_PROCS
    from concourse.vector_clock import ScopedClock, VectorClock

    def _drain_and_barrier(self, tick_clock, wait_clock):
        gc = tick_clock.global_clock
        for p in range(N_PROCS):
            t = gc[p]
            if t <= 0:
                continue
            ticks = [0] * N_PROCS
            ticks[p] = t
            di = self.nc.sync.drain()
            wait_clock.add_sem_waits(
                di.ins, ScopedClock({None: VectorClock(ticks)})
            )
        self.nc.all_engine_barrier()
        assert self.sems is not None
        popped = self.nc._tile_sem_poison_stack.pop()
        assert popped is self._sem_poison
        # Skip tile's per-range sem clear + second barrier: this is the
        # outermost tile at kernel end, and the NEFF epilogue already
        # restores the full semaphore file before the next execution. The
        # drains above have made SP wait out every DMA completion.

    tile_mod.TileContext._drain_and_barrier = _drain_and_barrier
    tile_mod.TileContext._drain_split_patched = True


def _build():
    import concourse.bass as bass
    import concourse.mybir as mybir
    import concourse.tile as tile

    _patch_drain_split()

    f32 = mybir.dt.float32
    bf16 = mybir.dt.bfloat16

    nc = bass.Bass()
    in_d = nc.dram_tensor("inp", [P, IN_COLS], bf16, kind="ExternalInput")
    z_d = nc.dram_tensor("z", [P, NG * B], bf16, kind="ExternalOutput")

    with tile.TileContext(nc) as tc:
        with (
            tc.tile_pool(name="const", bufs=1) as const,
            tc.tile_pool(name="psum", bufs=1, space="PSUM") as psum,
        ):
            inp = const.tile([P, IN_COLS], bf16)
            warm = const.tile([P, P], bf16)
            b32 = const.tile([P, NG], f32)        # bias columns, upcast
            dsc = const.tile([1, 8], f32)         # DVE absorber scratch
            tmps = [const.tile([P, B], bf16, name=f"tmp{g}", tag=f"tmp{g}")
                    for g in range(NG)]
            zsb = const.tile([P, NG * B], bf16)
            ps = [psum.tile([P, B], f32, name=f"ps{g}", tag=f"ps{g}")
                  for g in range(NG)]

            # --- in-DMAs: 5, issue order = FIFO stream order ---
            for lo, hi in ((0, D0_E), (D0_E, D1_E), (D1_E, D2_E),
                           (D2_E, D3_E), (D3_E, IN_COLS)):
                nc.sync.dma_start(inp[:, lo:hi], in_d[:, lo:hi])

            # --- PE warmup: lift the HAM clock gate while DMAs fill ---
            nc.vector.memset(warm[:], 1.0)
            for w in range(NW):
                nc.tensor.matmul(
                    ps[NG - 1][0:1, 0:P], warm[:, 0:1], warm[:, 0:P],
                    start=True, stop=True, skip_group_check=True,
                )

            # --- DVE: upcast bias, then cover its tick for later ops ---
            nc.vector.tensor_copy(b32[:], inp[:, BIAS_C:BIAS_C + NG])
            nc.vector.tensor_copy(dsc[0:1, 0:1], b32[0:1, 0:1])

            # --- PE absorbers: cover the two xT DMA sems ---
            def absorber(c):
                nc.tensor.matmul(
                    ps[NG - 1][0:1, 0:1], inp[:, c:c + 1], inp[:, c + 1:c + 2],
                    start=True, stop=True, skip_group_check=True,
                )

            absorber(XTA_C)        # D0 (also covers Wg0)
            for g in range(NG):
                last = g == NG - 1
                for i in range(NI):
                    if g == 0 and i == 4:
                        absorber(XTB_C)   # D1 (also covers ew/bias)
                    if last and i == NI - 1:
                        # split the final matmul into two column halves so
                        # the PSUM-bound bias-add of the first half runs
                        # under the second half instead of after it
                        for h in range(2):
                            nc.tensor.matmul(
                                ps[g][:, h * 256:(h + 1) * 256],
                                inp[:, _wcol(g, i):_wcol(g, i) + P],
                                inp[:, _xcol(i) + h * 256:_xcol(i) + (h + 1) * 256],
                                start=False, stop=True, skip_group_check=True,
                            )
                            nc.vector.tensor_tensor(
                                tmps[g][:, h * 256:(h + 1) * 256],
                                ps[g][:, h * 256:(h + 1) * 256],
                                b32[:, g:g + 1].broadcast_to([P, 256]),
                                mybir.AluOpType.add,
                            )
                        continue
                    nc.tensor.matmul(
                        ps[g][:],
                        inp[:, _wcol(g, i):_wcol(g, i) + P],
                        inp[:, _xcol(i):_xcol(i) + B],
                        start=(i == 0), stop=(i == NI - 1 and not last),
                    )
                # evict group g as soon as its accumulation closes:
                # bf16 tmp so the ew multiply runs in 16-bit 2x mode and
                # reads the ew broadcast straight out of the packed input
                if not last:
                    nc.vector.tensor_tensor(
                        tmps[g][:], ps[g][:],
                        b32[:, g:g + 1].broadcast_to([P, B]),
                        mybir.AluOpType.add,
                    )
                    nc.vector.tensor_tensor(
                        zsb[:, g * B:(g + 1) * B], tmps[g][:],
                        inp[:, EWB_C:EWB_C + B],
                        mybir.AluOpType.mult,
                    )
                else:
                    for h in range(2):
                        nc.vector.tensor_tensor(
                            zsb[:, g * B + h * 256:g * B + (h + 1) * 256],
                            tmps[g][:, h * 256:(h + 1) * 256],
                            inp[:, EWB_C + h * 256:EWB_C + (h + 1) * 256],
                            mybir.AluOpType.mult,
                        )
                if g in (3, 6, 7):
                    lo = 0 if g == 3 else (4 * B if g == 6 else 7 * B)
                    hi = (g + 1) * B
                    nc.sync.dma_start(z_d[:, lo:hi], zsb[:, lo:hi])

    return nc


def _get_compiled():
    global _compiled
    if _compiled is None:
        _compiled = _build()
    return _compiled


_prep_cache = None


def _make_in_maps(x, expert_weights, weight, bias):
    global _prep_cache
    import ml_dtypes

    bf = ml_dtypes.bfloat16
    if _prep_cache is None or _prep_cache[0] is not weight:
        wt = np.asarray(weight, dtype=np.float32)
        # wg[p, (g,i,c)] = W[k, g*128+c, i*128+p]
        wgs = [
            wt[k].T.reshape(NI, P, NG, P)
            .transpose(1, 2, 0, 3).reshape(P, NG * NI * P).astype(bf)
            for k in range(NCORES)
        ]
        _prep_cache = (weight, wgs)
    wgs = _prep_cache[1]
    # xT chunks: xt[p, i*512+b] = x[b, i*128+p] — same bytes every core
    xt = (np.asarray(x, dtype=np.float32).T.reshape(NI, P, B)
          .transpose(1, 0, 2).reshape(P, NI * B).astype(bf))
    ew = np.asarray(expert_weights, dtype=np.float32)
    bs = np.asarray(bias, dtype=np.float32)
    in_maps = []
    for k in range(NCORES):
        inp = np.empty((P, IN_COLS), dtype=bf)
        inp[:, XTA_C:XTA_C + 4 * B] = xt[:, :4 * B]
        inp[:, XTB_C:XTB_C + 4 * B] = xt[:, 4 * B:]
        inp[:, WG0_C:WG0_C + NI * P] = wgs[k][:, :NI * P]
        inp[:, WGR_C:IN_COLS] = wgs[k][:, NI * P:]
        inp[:, EWB_C:EWB_C + B] = np.broadcast_to(ew[:, k], (P, B))
        inp[:, BIAS_C:BIAS_C + NG] = bs[k].reshape(NG, P).T
        in_maps.append({"inp": inp})
    return in_maps


def kernel(x, expert_weights, weight, bias, _trace=False):
    from concourse.bass_utils import run_bass_kernel_spmd

    nc = _get_compiled()
    in_maps = _make_in_maps(x, expert_weights, weight, bias)
    res = run_bass_kernel_spmd(
        nc, in_maps, core_ids=list(range(NCORES)), trace=_trace
    )
    y = np.zeros((B, OUT), dtype=np.float32)
    for r in res.results:
        z = np.asarray(r["z"]).astype(np.float32)  # [128, 8*512]
        y += z.reshape(P, NG, B).transpose(1, 0, 2).reshape(OUT, B).T
    if _trace:
        return y, res
    return y


# revision 18
# speedup vs baseline: 1.1000x; 1.1000x over previous
"""ExpertLinear (dense MoE blend) Trainium2 kernel — expert-parallel.

y[b,o] = sum_k ew[b,k] * (x[b,:] @ W[k,o,:] + bias[k,o])

Sharding: expert-parallel across 8 cores (core k owns expert k). Each core
computes the full partial z_k = ew[:,k] * (x @ W[k].T + bias[k]) for ALL
512 rows; the host unshard step is a pure sum of the 8 partials. Per-core
HBM traffic is ~4.3 MB (W_k 2 MB bf16 + xT 1 MB bf16 + z_k 1 MB bf16 out)
vs 18.6 MB for the data-parallel layout, because each expert's weights are
read exactly once chip-wide.

Device flow per core (all operands packed in ONE bf16 dram tensor):
  - PE: per o-block group g (8 of them), 8 matmuls (lhsT = W chunk
    [128i,128o] stationary, rhs = xT chunk [128i,512b] moving) accumulate
    zT_k group [128o,512b] into PSUM bank g; all 8 banks live at once,
    each evicted as soon as its group closes so only the last eviction is
    exposed.
  - DVE evict: z = (ps + bias_col) * ew_bcast, downcast bf16. The ew
    blend and bias stay on device; the host only sums partials.
  - Walrus accepts ONE sync wait per instruction and tile emits a sem
    wait for EVERY data dep (even same-engine), so: absorber matmuls
    cover each in-DMA's sem on the PE queue (later matmuls on the same
    sem are coverage-deduped), a tiny DVE copy covers the ew/bias load,
    per-group tmp buffers kill the evict WAR chain, and the DMA count is
    capped at 8 (5 in + 3 out) so no DMAHW sem lane is ever reused (a
    reused lane adds a second wait to that DMA).
  - All DMAs ride the single qSPDynamicHW FIFO ring: the 5 in-DMAs
    stream back-to-back at full bandwidth in issue order, sized so
    compute can start after ~0.8 MB; out-DMAs queue behind them and
    never steal in-stream bandwidth. ~20 warmup matmuls run during the
    initial DMA fill to lift the PE HAM clock gate (1.2 -> 2.4 GHz).
"""

import numpy as np

B, E, IN, OUT = 512, 8, 1024, 1024
NCORES = 8
P = 128
NI = IN // P    # 8 i-chunks (contraction)
NG = OUT // P   # 8 o-block groups
NW = 36         # warmup matmuls (bridge PE from preamble exit to first W arrival;
                # must overshoot D0 arrival slightly — an early gap re-throttles HAM)

# input column layout (bf16, [128, IN_COLS])
XTA_C = 0                     # xT chunks 0-3: [128, 4*512]
WG0_C = XTA_C + 4 * B         # W group 0: [128, 8*128]
XTB_C = WG0_C + NI * P        # xT chunks 4-7: [128, 4*512]
EWB_C = XTB_C + 4 * B         # ew column broadcast: [128, 512]
BIAS_C = EWB_C + B            # biasT: [128, 8]
WGR_C = BIAS_C + NG           # W groups 1-7: [128, 7*8*128]
IN_COLS = WGR_C + (NG - 1) * NI * P

# in-DMA boundaries, cut so each segment lands just before PE needs it:
# [xTa|Wg0], [xTb|ewb|biasT], [Wg1], [Wg2|Wg3], [Wg4-7]
D0_E = XTB_C
D1_E = WGR_C
D2_E = WGR_C + NI * P
D3_E = WGR_C + 3 * NI * P


def _wcol(g, i):
    base = WG0_C if g == 0 else WGR_C + (g - 1) * NI * P
    return base + i * P


def _xcol(i):
    return (XTA_C if i < 4 else XTB_C - 4 * B) + i * B


_compiled = None


def _patch_drain_split():
    """The walrus build in this container rejects any instruction carrying
    more than one sync wait, including the kernel-tail Drain that
    TileContext emits with one wait per active semaphore. Split it into a
    sequence of single-wait drains (sequencer-FIFO keeps them ordered;
    the set of waits is identical)."""
    import concourse.tile as tile_mod

    if getattr(tile_mod.TileContext, "_drain_split_patched", False):
        return
    from concourse.tile_sem_assignment import N_PROCS
    from concourse.vector_clock import ScopedClock, VectorClock

    def _drain_and_barrier(self, tick_clock, wait_clock):
        gc = tick_clock.global_clock
        for p in range(N_PROCS):
            t = gc[p]
            if t <= 0:
                continue
            ticks = [0] * N_PROCS
            ticks[p] = t
            di = self.nc.sync.drain()
            wait_clock.add_sem_waits(
                di.ins, ScopedClock({None: VectorClock(ticks)})
            )
        self.nc.all_engine_barrier()
        assert self.sems is not None
        popped = self.nc._tile_sem_poison_stack.pop()
        assert popped is self._sem_poison
        # Skip tile's per-range sem clear + second barrier: this is the
        # outermost tile at kernel end, and the NEFF epilogue already
        # restores the full semaphore file before the next execution. The
        # drains above have made SP wait out every DMA completion.

    tile_mod.TileContext._drain_and_barrier = _drain_and_barrier
    tile_mod.TileContext._drain_split_patched = True


def _build():
    import concourse.bass as bass
    import concourse.mybir as mybir
    import concourse.tile as tile

    _patch_drain_split()

    f32 = mybir.dt.float32
    bf16 = mybir.dt.bfloat16

    nc = bass.Bass()
    in_d = nc.dram_tensor("inp", [P, IN_COLS], bf16, kind="ExternalInput")
    z_d = nc.dram_tensor("z", [P, NG * B], bf16, kind="ExternalOutput")

    with tile.TileContext(nc) as tc:
        with (
            tc.tile_pool(name="const", bufs=1) as const,
            tc.tile_pool(name="psum", bufs=1, space="PSUM") as psum,
        ):
            inp = const.tile([P, IN_COLS], bf16)
            warm = const.tile([P, P], bf16)
            b32 = const.tile([P, NG], f32)        # bias columns, upcast
            dsc = const.tile([1, 8], f32)         # DVE absorber scratch
            tmps = [const.tile([P, B], bf16, name=f"tmp{g}", tag=f"tmp{g}")
                    for g in range(NG)]
            zsb = const.tile([P, NG * B], bf16)
            ps = [psum.tile([P, B], f32, name=f"ps{g}", tag=f"ps{g}")
                  for g in range(NG)]

            # --- in-DMAs: 5, issue order = FIFO stream order ---
            for lo, hi in ((0, D0_E), (D0_E, D1_E), (D1_E, D2_E),
                           (D2_E, D3_E), (D3_E, IN_COLS)):
                nc.sync.dma_start(inp[:, lo:hi], in_d[:, lo:hi])

            # --- PE warmup: lift the HAM clock gate while DMAs fill ---
            nc.vector.memset(warm[:], 1.0)
            for w in range(NW):
                nc.tensor.matmul(
                    ps[NG - 1][0:1, 0:P], warm[:, 0:1], warm[:, 0:P],
                    start=True, stop=True, skip_group_check=True,
                )

            # --- DVE: upcast bias, then cover its tick for later ops ---
            nc.vector.tensor_copy(b32[:], inp[:, BIAS_C:BIAS_C + NG])
            nc.vector.tensor_copy(dsc[0:1, 0:1], b32[0:1, 0:1])

            # --- PE absorbers: cover the two xT DMA sems ---
            def absorber(c):
                nc.tensor.matmul(
                    ps[NG - 1][0:1, 0:1], inp[:, c:c + 1], inp[:, c + 1:c + 2],
                    start=True, stop=True, skip_group_check=True,
                )

            absorber(XTA_C)        # D0 (also covers Wg0)
            for g in range(NG):
                last = g == NG - 1
                for i in range(NI):
                    if g == 0 and i == 4:
                        absorber(XTB_C)   # D1 (also covers ew/bias)
                    if last and i == NI - 1:
                        # split the final matmul into two column halves so
                        # the PSUM-bound bias-add of the first half runs
                        # under the second half instead of after it
                        for h in range(2):
                            nc.tensor.matmul(
                                ps[g][:, h * 256:(h + 1) * 256],
                                inp[:, _wcol(g, i):_wcol(g, i) + P],
                                inp[:, _xcol(i) + h * 256:_xcol(i) + (h + 1) * 256],
                                start=False, stop=True, skip_group_check=True,
                            )
                            nc.vector.tensor_tensor(
                                tmps[g][:, h * 256:(h + 1) * 256],
                                ps[g][:, h * 256:(h + 1) * 256],
                                b32[:, g:g + 1].broadcast_to([P, 256]),
                                mybir.AluOpType.add,
                            )
                        continue
                    nc.tensor.matmul(
                        ps[g][:],
                        inp[:, _wcol(g, i):_wcol(g, i) + P],
                        inp[:, _xcol(i):_xcol(i) + B],
                        start=(i == 0), stop=(i == NI - 1 and not last),
                    )
                # evict group g as soon as its accumulation closes:
                # bf16 tmp so the ew multiply runs in 16-bit 2x mode and
                # reads the ew broadcast straight out of the packed input
                if not last:
                    nc.vector.tensor_tensor(
                        tmps[g][:], ps[g][:],
                        b32[:, g:g + 1].broadcast_to([P, B]),
                        mybir.AluOpType.add,
                    )
                    nc.vector.tensor_tensor(
                        zsb[:, g * B:(g + 1) * B], tmps[g][:],
                        inp[:, EWB_C:EWB_C + B],
                        mybir.AluOpType.mult,
                    )
                else:
                    for h in range(2):
                        nc.vector.tensor_tensor(
                            zsb[:, g * B + h * 256:g * B + (h + 1) * 256],
                            tmps[g][:, h * 256:(h + 1) * 256],
                            inp[:, EWB_C + h * 256:EWB_C + (h + 1) * 256],
                            mybir.AluOpType.mult,
                        )
                if g in (3, 6, 7):
                    lo = 0 if g == 3 else (4 * B if g == 6 else 7 * B)
                    hi = (g + 1) * B
                    nc.sync.dma_start(z_d[:, lo:hi], zsb[:, lo:hi])

    return nc


def _get_compiled():
    global _compiled
    if _compiled is None:
        _compiled = _build()
    return _compiled


_prep_cache = None


def _make_in_maps(x, expert_weights, weight, bias):
    global _prep_cache
    import ml_dtypes

    bf = ml_dtypes.bfloat16
    if _prep_cache is None or _prep_cache[0] is not weight:
        wt = np.asarray(weight, dtype=np.float32)
        # wg[p, (g,i,c)] = W[k, g*128+c, i*128+p]
        wgs = [
            wt[k].T.reshape(NI, P, NG, P)
            .transpose(1, 2, 0, 3).reshape(P, NG * NI * P).astype(bf)
            for k in range(NCORES)
        ]
        _prep_cache = (weight, wgs)
    wgs = _prep_cache[1]
    # xT chunks: xt[p, i*512+b] = x[b, i*128+p] — same bytes every core
    xt = (np.asarray(x, dtype=np.float32).T.reshape(NI, P, B)
          .transpose(1, 0, 2).reshape(P, NI * B).astype(bf))
    ew = np.asarray(expert_weights, dtype=np.float32)
    bs = np.asarray(bias, dtype=np.float32)
    in_maps = []
    for k in range(NCORES):
        inp = np.empty((P, IN_COLS), dtype=bf)
        inp[:, XTA_C:XTA_C + 4 * B] = xt[:, :4 * B]
        inp[:, XTB_C:XTB_C + 4 * B] = xt[:, 4 * B:]
        inp[:, WG0_C:WG0_C + NI * P] = wgs[k][:, :NI * P]
        inp[:, WGR_C:IN_COLS] = wgs[k][:, NI * P:]
        inp[:, EWB_C:EWB_C + B] = np.broadcast_to(ew[:, k], (P, B))
        inp[:, BIAS_C:BIAS_C + NG] = bs[k].reshape(NG, P).T
        in_maps.append({"inp": inp})
    return in_maps


def kernel(x, expert_weights, weight, bias, _trace=False):
    from concourse.bass_utils import run_bass_kernel_spmd

    nc = _get_compiled()
    in_maps = _make_in_maps(x, expert_weights, weight, bias)
    res = run_bass_kernel_spmd(
        nc, in_maps, core_ids=list(range(NCORES)), trace=_trace
    )
    y = np.zeros((B, OUT), dtype=np.float32)
    for r in res.results:
        z = np.asarray(r["z"]).astype(np.float32)  # [128, 8*512]
        y += z.reshape(P, NG, B).transpose(1, 0, 2).reshape(OUT, B).T
    if _trace:
        return y, res
    return y


# revision 20
# speedup vs baseline: 1.1215x; 1.0195x over previous
"""ExpertLinear (dense MoE blend) Trainium2 kernel — expert-parallel.

y[b,o] = sum_k ew[b,k] * (x[b,:] @ W[k,o,:] + bias[k,o])

Sharding: expert-parallel across 8 cores (core k owns expert k). Each core
computes the full partial z_k = ew[:,k] * (x @ W[k].T + bias[k]) for ALL
512 rows; the host unshard step is a pure sum of the 8 partials. Per-core
HBM traffic is ~4.3 MB (W_k 2 MB bf16 + xT 1 MB bf16 + z_k 1 MB bf16 out)
vs 18.6 MB for the data-parallel layout, because each expert's weights are
read exactly once chip-wide.

Device flow per core (all operands packed in ONE bf16 dram tensor):
  - PE: per o-block group g (8 of them), 8 matmuls (lhsT = W chunk
    [128i,128o] stationary, rhs = xT chunk [128i,512b] moving) accumulate
    zT_k group [128o,512b] into PSUM bank g; all 8 banks live at once,
    each evicted as soon as its group closes so only the last eviction is
    exposed.
  - DVE evict: z = (ps + bias_col) * ew_bcast, downcast bf16. The ew
    blend and bias stay on device; the host only sums partials.
  - Walrus accepts ONE sync wait per instruction and tile emits a sem
    wait for EVERY data dep (even same-engine), so: absorber matmuls
    cover each in-DMA's sem on the PE queue (later matmuls on the same
    sem are coverage-deduped), a tiny DVE copy covers the ew/bias load,
    per-group tmp buffers kill the evict WAR chain, and the DMA count is
    capped at 8 (5 in + 3 out) so no DMAHW sem lane is ever reused (a
    reused lane adds a second wait to that DMA).
  - All DMAs ride the single qSPDynamicHW FIFO ring: the 5 in-DMAs
    stream back-to-back at full bandwidth in issue order, sized so
    compute can start after ~0.8 MB; out-DMAs queue behind them and
    never steal in-stream bandwidth. ~20 warmup matmuls run during the
    initial DMA fill to lift the PE HAM clock gate (1.2 -> 2.4 GHz).
"""

import numpy as np

B, E, IN, OUT = 512, 8, 1024, 1024
NCORES = 8
P = 128
NI = IN // P    # 8 i-chunks (contraction)
NG = OUT // P   # 8 o-block groups
NW = 36         # warmup matmuls (bridge PE from preamble exit to first W arrival;
                # must overshoot D0 arrival slightly — an early gap re-throttles HAM)

# input column layout (bf16, [128, IN_COLS])
XTA_C = 0                     # xT chunks 0-3: [128, 4*512]
WG0_C = XTA_C + 4 * B         # W group 0: [128, 8*128]
XTB_C = WG0_C + NI * P        # xT chunks 4-7: [128, 4*512]
EWB_C = XTB_C + 4 * B         # ew column broadcast: [128, 512]
BIAS_C = EWB_C + B            # biasT: [128, 8]
WGR_C = BIAS_C + NG           # W groups 1-7: [128, 7*8*128]
IN_COLS = WGR_C + (NG - 1) * NI * P

# in-DMA boundaries, cut so each segment lands just before PE needs it:
# [xTa|Wg0], [xTb], [ewb|biasT|Wg1], [Wg2|Wg3], [Wg4-7]
D0_E = XTB_C
D1_E = EWB_C
D2_E = WGR_C + NI * P
D3_E = WGR_C + 3 * NI * P


def _wcol(g, i):
    base = WG0_C if g == 0 else WGR_C + (g - 1) * NI * P
    return base + i * P


def _xcol(i):
    return (XTA_C if i < 4 else XTB_C - 4 * B) + i * B


_compiled = None


def _patch_drain_split():
    """The walrus build in this container rejects any instruction carrying
    more than one sync wait, including the kernel-tail Drain that
    TileContext emits with one wait per active semaphore. Split it into a
    sequence of single-wait drains (sequencer-FIFO keeps them ordered;
    the set of waits is identical)."""
    import concourse.tile as tile_mod

    if getattr(tile_mod.TileContext, "_drain_split_patched", False):
        return
    from concourse.tile_sem_assignment import N_PROCS
    from concourse.vector_clock import ScopedClock, VectorClock

    def _drain_and_barrier(self, tick_clock, wait_clock):
        gc = tick_clock.global_clock
        for p in range(N_PROCS):
            t = gc[p]
            if t <= 0:
                continue
            ticks = [0] * N_PROCS
            ticks[p] = t
            di = self.nc.sync.drain()
            wait_clock.add_sem_waits(
                di.ins, ScopedClock({None: VectorClock(ticks)})
            )
        assert self.sems is not None
        popped = self.nc._tile_sem_poison_stack.pop()
        assert popped is self._sem_poison
        # Skip tile's exit barriers + per-range sem clear: this is the
        # outermost tile at kernel end; the NEFF epilogue has its own
        # all-engine barrier before it restores the full semaphore file,
        # and the drains above make SP wait out every DMA completion
        # before it reaches that barrier.

    tile_mod.TileContext._drain_and_barrier = _drain_and_barrier
    tile_mod.TileContext._drain_split_patched = True


def _build():
    import concourse.bass as bass
    import concourse.mybir as mybir
    import concourse.tile as tile

    _patch_drain_split()

    f32 = mybir.dt.float32
    bf16 = mybir.dt.bfloat16

    nc = bass.Bass()
    in_d = nc.dram_tensor("inp", [P, IN_COLS], bf16, kind="ExternalInput")
    z_d = nc.dram_tensor("z", [P, NG * B], bf16, kind="ExternalOutput")

    with tile.TileContext(nc) as tc:
        with (
            tc.tile_pool(name="const", bufs=1) as const,
            tc.tile_pool(name="psum", bufs=1, space="PSUM") as psum,
        ):
            inp = const.tile([P, IN_COLS], bf16)
            warm = const.tile([P, P], bf16)
            b32 = const.tile([P, NG], f32)        # bias columns, upcast
            dsc = const.tile([1, 8], f32)         # DVE absorber scratch
            tmps = [const.tile([P, B], bf16, name=f"tmp{g}", tag=f"tmp{g}")
                    for g in range(NG)]
            zsb = const.tile([P, NG * B], bf16)
            ps = [psum.tile([P, B], f32, name=f"ps{g}", tag=f"ps{g}")
                  for g in range(NG)]

            # --- in-DMAs: 5, issue order = FIFO stream order ---
            for lo, hi in ((0, D0_E), (D0_E, D1_E), (D1_E, D2_E),
                           (D2_E, D3_E), (D3_E, IN_COLS)):
                nc.sync.dma_start(inp[:, lo:hi], in_d[:, lo:hi])

            # --- PE warmup: lift the HAM clock gate while DMAs fill ---
            nc.vector.memset(warm[:], 1.0)
            for w in range(NW):
                nc.tensor.matmul(
                    ps[NG - 1][0:1, 0:P], warm[:, 0:1], warm[:, 0:P],
                    start=True, stop=True, skip_group_check=True,
                )

            # --- DVE: upcast bias, then cover its tick for later ops ---
            nc.vector.tensor_copy(b32[:], inp[:, BIAS_C:BIAS_C + NG])
            nc.vector.tensor_copy(dsc[0:1, 0:1], b32[0:1, 0:1])

            # --- PE absorbers: cover the two xT DMA sems ---
            def absorber(c):
                nc.tensor.matmul(
                    ps[NG - 1][0:1, 0:1], inp[:, c:c + 1], inp[:, c + 1:c + 2],
                    start=True, stop=True, skip_group_check=True,
                )

            absorber(XTA_C)        # D0 (also covers Wg0)
            for g in range(NG):
                last = g == NG - 1
                for i in range(NI):
                    if g == 0 and i == 4:
                        absorber(XTB_C)   # D1 (also covers ew/bias)
                    if last and i == NI - 1:
                        # split the final matmul into two column halves so
                        # the PSUM-bound bias-add of the first half runs
                        # under the second half instead of after it
                        for h in range(2):
                            nc.tensor.matmul(
                                ps[g][:, h * 256:(h + 1) * 256],
                                inp[:, _wcol(g, i):_wcol(g, i) + P],
                                inp[:, _xcol(i) + h * 256:_xcol(i) + (h + 1) * 256],
                                start=False, stop=True, skip_group_check=True,
                            )
                            nc.vector.tensor_tensor(
                                tmps[g][:, h * 256:(h + 1) * 256],
                                ps[g][:, h * 256:(h + 1) * 256],
                                b32[:, g:g + 1].broadcast_to([P, 256]),
                                mybir.AluOpType.add,
                            )
                        continue
                    nc.tensor.matmul(
                        ps[g][:],
                        inp[:, _wcol(g, i):_wcol(g, i) + P],
                        inp[:, _xcol(i):_xcol(i) + B],
                        start=(i == 0), stop=(i == NI - 1 and not last),
                    )
                # evict group g as soon as its accumulation closes:
                # bf16 tmp so the ew multiply runs in 16-bit 2x mode and
                # reads the ew broadcast straight out of the packed input
                if not last:
                    nc.vector.tensor_tensor(
                        tmps[g][:], ps[g][:],
                        b32[:, g:g + 1].broadcast_to([P, B]),
                        mybir.AluOpType.add,
                    )
                    nc.vector.tensor_tensor(
                        zsb[:, g * B:(g + 1) * B], tmps[g][:],
                        inp[:, EWB_C:EWB_C + B],
                        mybir.AluOpType.mult,
                    )
                else:
                    for h in range(2):
                        nc.vector.tensor_tensor(
                            zsb[:, g * B + h * 256:g * B + (h + 1) * 256],
                            tmps[g][:, h * 256:(h + 1) * 256],
                            inp[:, EWB_C + h * 256:EWB_C + (h + 1) * 256],
                            mybir.AluOpType.mult,
                        )
                if g in (3, 6, 7):
                    lo = 0 if g == 3 else (4 * B if g == 6 else 7 * B)
                    hi = (g + 1) * B
                    nc.sync.dma_start(z_d[:, lo:hi], zsb[:, lo:hi])

    return nc


def _get_compiled():
    global _compiled
    if _compiled is None:
        _compiled = _build()
    return _compiled


_prep_cache = None


def _make_in_maps(x, expert_weights, weight, bias):
    global _prep_cache
    import ml_dtypes

    bf = ml_dtypes.bfloat16
    if _prep_cache is None or _prep_cache[0] is not weight:
        wt = np.asarray(weight, dtype=np.float32)
        # wg[p, (g,i,c)] = W[k, g*128+c, i*128+p]
        wgs = [
            wt[k].T.reshape(NI, P, NG, P)
            .transpose(1, 2, 0, 3).reshape(P, NG * NI * P).astype(bf)
            for k in range(NCORES)
        ]
        _prep_cache = (weight, wgs)
    wgs = _prep_cache[1]
    # xT chunks: xt[p, i*512+b] = x[b, i*128+p] — same bytes every core
    xt = (np.asarray(x, dtype=np.float32).T.reshape(NI, P, B)
          .transpose(1, 0, 2).reshape(P, NI * B).astype(bf))
    ew = np.asarray(expert_weights, dtype=np.float32)
    bs = np.asarray(bias, dtype=np.float32)
    in_maps = []
    for k in range(NCORES):
        inp = np.empty((P, IN_COLS), dtype=bf)
        inp[:, XTA_C:XTA_C + 4 * B] = xt[:, :4 * B]
        inp[:, XTB_C:XTB_C + 4 * B] = xt[:, 4 * B:]
        inp[:, WG0_C:WG0_C + NI * P] = wgs[k][:, :NI * P]
        inp[:, WGR_C:IN_COLS] = wgs[k][:, NI * P:]
        inp[:, EWB_C:EWB_C + B] = np.broadcast_to(ew[:, k], (P, B))
        inp[:, BIAS_C:BIAS_C + NG] = bs[k].reshape(NG, P).T
        in_maps.append({"inp": inp})
    return in_maps


def kernel(x, expert_weights, weight, bias, _trace=False):
    from concourse.bass_utils import run_bass_kernel_spmd

    nc = _get_compiled()
    in_maps = _make_in_maps(x, expert_weights, weight, bias)
    res = run_bass_kernel_spmd(
        nc, in_maps, core_ids=list(range(NCORES)), trace=_trace
    )
    y = np.zeros((B, OUT), dtype=np.float32)
    for r in res.results:
        z = np.asarray(r["z"]).astype(np.float32)  # [128, 8*512]
        y += z.reshape(P, NG, B).transpose(1, 0, 2).reshape(OUT, B).T
    if _trace:
        return y, res
    return y


# revision 23
# speedup vs baseline: 1.1563x; 1.0311x over previous
"""ExpertLinear (dense MoE blend) Trainium2 kernel — expert-parallel.

y[b,o] = sum_k ew[b,k] * (x[b,:] @ W[k,o,:] + bias[k,o])

Sharding: expert-parallel across 8 cores (core k owns expert k). Each core
computes the full partial z_k = ew[:,k] * (x @ W[k].T + bias[k]) for ALL
512 rows; the host unshard step is a pure sum of the 8 partials. Per-core
HBM traffic is ~4.3 MB (W_k 2 MB bf16 + xT 1 MB bf16 + z_k 1 MB bf16 out)
vs 18.6 MB for the data-parallel layout, because each expert's weights are
read exactly once chip-wide.

Device flow per core (all operands packed in ONE bf16 dram tensor):
  - PE: per o-block group g (8 of them), 8 matmuls (lhsT = W chunk
    [128i,128o] stationary, rhs = xT chunk [128i,512b] moving) accumulate
    zT_k group [128o,512b] into PSUM bank g; all 8 banks live at once,
    each evicted as soon as its group closes so only the last eviction is
    exposed.
  - DVE evict: z = (ps + bias_col) * ew_bcast, downcast bf16. The ew
    blend and bias stay on device; the host only sums partials.
  - Walrus accepts ONE sync wait per instruction and tile emits a sem
    wait for EVERY data dep (even same-engine), so: absorber matmuls
    cover each in-DMA's sem on the PE queue (later matmuls on the same
    sem are coverage-deduped), a tiny DVE copy covers the ew/bias load,
    per-group tmp buffers kill the evict WAR chain, and the DMA count is
    capped at 8 (5 in + 3 out) so no DMAHW sem lane is ever reused (a
    reused lane adds a second wait to that DMA).
  - All DMAs ride the single qSPDynamicHW FIFO ring: the 5 in-DMAs
    stream back-to-back at full bandwidth in issue order, sized so
    compute can start after ~0.8 MB; out-DMAs queue behind them and
    never steal in-stream bandwidth. ~20 warmup matmuls run during the
    initial DMA fill to lift the PE HAM clock gate (1.2 -> 2.4 GHz).
"""

import numpy as np

B, E, IN, OUT = 512, 8, 1024, 1024
NCORES = 8
P = 128
NI = IN // P    # 8 i-chunks (contraction)
NG = OUT // P   # 8 o-block groups
NW = 36         # warmup matmuls (bridge PE from preamble exit to first W arrival;
                # must overshoot D0 arrival slightly — an early gap re-throttles HAM)

# input column layout (bf16, [128, IN_COLS])
XTA_C = 0                     # xT chunks 0-3: [128, 4*512]
WG0_C = XTA_C + 4 * B         # W group 0: [128, 8*128]
XTB_C = WG0_C + NI * P        # xT chunks 4-7: [128, 4*512]
EWB_C = XTB_C + 4 * B         # ew column broadcast: [128, 512]
BIAS_C = EWB_C + B            # biasT: [128, 8]
WGR_C = BIAS_C + NG           # W groups 1-7: [128, 7*8*128]
IN_COLS = WGR_C + (NG - 1) * NI * P

# in-DMA boundaries, cut so each segment lands just before PE needs it:
# [xTa|Wg0], [xTb], [ewb|biasT|Wg1], [Wg2|Wg3], [Wg4-7]
D0_E = XTB_C
D1_E = EWB_C
D2_E = WGR_C + NI * P
D3_E = WGR_C + 3 * NI * P


def _wcol(g, i):
    base = WG0_C if g == 0 else WGR_C + (g - 1) * NI * P
    return base + i * P


def _xcol(i):
    return (XTA_C if i < 4 else XTB_C - 4 * B) + i * B


_compiled = None


def _patch_drain_split():
    """The walrus build in this container rejects any instruction carrying
    more than one sync wait, including the kernel-tail Drain that
    TileContext emits with one wait per active semaphore. Split it into a
    sequence of single-wait drains (sequencer-FIFO keeps them ordered;
    the set of waits is identical)."""
    import concourse.tile as tile_mod

    if getattr(tile_mod.TileContext, "_drain_split_patched", False):
        return
    from concourse.tile_sem_assignment import N_PROCS
    from concourse.vector_clock import ScopedClock, VectorClock

    def _drain_and_barrier(self, tick_clock, wait_clock):
        gc = tick_clock.global_clock
        for p in range(N_PROCS):
            t = gc[p]
            if t <= 0:
                continue
            ticks = [0] * N_PROCS
            ticks[p] = t
            di = self.nc.sync.drain()
            wait_clock.add_sem_waits(
                di.ins, ScopedClock({None: VectorClock(ticks)})
            )
        assert self.sems is not None
        popped = self.nc._tile_sem_poison_stack.pop()
        assert popped is self._sem_poison
        # Skip tile's exit barriers + per-range sem clear: this is the
        # outermost tile at kernel end; the NEFF epilogue has its own
        # all-engine barrier before it restores the full semaphore file,
        # and the drains above make SP wait out every DMA completion
        # before it reaches that barrier.

    tile_mod.TileContext._drain_and_barrier = _drain_and_barrier
    tile_mod.TileContext._drain_split_patched = True


def _patch_skip_const_aps():
    """Bass.__init__ unconditionally memsets four const APs (0.0/1.0/...)
    and runs an all-engine barrier before the kernel body. This kernel
    never consumes a const AP (no immediates/iota), and those ops sit at
    the head of the measured window gating the first DMA issue. Skip
    emitting them (the APs are still registered; their SBUF bytes are
    simply never initialized or read)."""
    import concourse.bass as bass_mod

    if getattr(bass_mod.Bass, "_const_ap_skip_patched", False):
        return
    orig_init = bass_mod.Bass.__init__
    orig_barrier = bass_mod.Bass.all_engine_barrier

    def new_init(self, *a, **k):
        self._in_const_skip_init = True
        try:
            orig_init(self, *a, **k)
        finally:
            self._in_const_skip_init = False

    def new_barrier(self, *a, **k):
        if getattr(self, "_in_const_skip_init", False):
            return None
        return orig_barrier(self, *a, **k)

    bass_mod.Bass.__init__ = new_init
    bass_mod.Bass.all_engine_barrier = new_barrier

    # gpsimd.memset during __init__ = exactly the four const-AP fills
    eng_memset = bass_mod.BassEitherVectorEngine.memset

    def new_memset(self, ap, constant):
        b = getattr(self, "bass", None)
        if b is not None and getattr(b, "_in_const_skip_init", False):
            return None
        return eng_memset(self, ap, constant)

    bass_mod.BassEitherVectorEngine.memset = new_memset
    bass_mod.Bass._const_ap_skip_patched = True


def _build():
    import concourse.bass as bass
    import concourse.mybir as mybir
    import concourse.tile as tile

    _patch_drain_split()
    _patch_skip_const_aps()

    f32 = mybir.dt.float32
    bf16 = mybir.dt.bfloat16

    nc = bass.Bass()
    in_d = nc.dram_tensor("inp", [P, IN_COLS], bf16, kind="ExternalInput")
    z_d = nc.dram_tensor("z", [P, NG * B], bf16, kind="ExternalOutput")

    with tile.TileContext(nc) as tc:
        with (
            tc.tile_pool(name="const", bufs=1) as const,
            tc.tile_pool(name="psum", bufs=1, space="PSUM") as psum,
        ):
            inp = const.tile([P, IN_COLS], bf16)
            warm = const.tile([P, P], bf16)
            b32 = const.tile([P, NG], f32)        # bias columns, upcast
            dsc = const.tile([1, 8], f32)         # DVE absorber scratch
            tmps = [const.tile([P, B], bf16, name=f"tmp{g}", tag=f"tmp{g}")
                    for g in range(NG)]
            zsb = const.tile([P, NG * B], bf16)
            ps = [psum.tile([P, B], f32, name=f"ps{g}", tag=f"ps{g}")
                  for g in range(NG)]

            # --- in-DMAs: 5, issue order = FIFO stream order ---
            for lo, hi in ((0, D0_E), (D0_E, D1_E), (D1_E, D2_E),
                           (D2_E, D3_E), (D3_E, IN_COLS)):
                nc.sync.dma_start(inp[:, lo:hi], in_d[:, lo:hi])

            # --- PE warmup: lift the HAM clock gate while DMAs fill ---
            nc.vector.memset(warm[:], 1.0)
            for w in range(NW):
                nc.tensor.matmul(
                    ps[NG - 1][0:1, 0:P], warm[:, 0:1], warm[:, 0:P],
                    start=True, stop=True, skip_group_check=True,
                )

            # --- DVE: upcast bias, then cover its tick for later ops ---
            nc.vector.tensor_copy(b32[:], inp[:, BIAS_C:BIAS_C + NG])
            nc.vector.tensor_copy(dsc[0:1, 0:1], b32[0:1, 0:1])

            # --- PE absorbers: cover the two xT DMA sems ---
            def absorber(c):
                nc.tensor.matmul(
                    ps[NG - 1][0:1, 0:1], inp[:, c:c + 1], inp[:, c + 1:c + 2],
                    start=True, stop=True, skip_group_check=True,
                )

            absorber(XTA_C)        # D0 (also covers Wg0)
            for g in range(NG):
                last = g == NG - 1
                for i in range(NI):
                    if g == 0 and i == 4:
                        absorber(XTB_C)   # D1 (also covers ew/bias)
                    if last and i == NI - 1:
                        # split the final matmul into two column halves so
                        # the PSUM-bound bias-add of the first half runs
                        # under the second half instead of after it
                        for h in range(2):
                            nc.tensor.matmul(
                                ps[g][:, h * 256:(h + 1) * 256],
                                inp[:, _wcol(g, i):_wcol(g, i) + P],
                                inp[:, _xcol(i) + h * 256:_xcol(i) + (h + 1) * 256],
                                start=False, stop=True, skip_group_check=True,
                            )
                            nc.vector.tensor_tensor(
                                tmps[g][:, h * 256:(h + 1) * 256],
                                ps[g][:, h * 256:(h + 1) * 256],
                                b32[:, g:g + 1].broadcast_to([P, 256]),
                                mybir.AluOpType.add,
                            )
                        continue
                    nc.tensor.matmul(
                        ps[g][:],
                        inp[:, _wcol(g, i):_wcol(g, i) + P],
                        inp[:, _xcol(i):_xcol(i) + B],
                        start=(i == 0), stop=(i == NI - 1 and not last),
                    )
                # evict group g as soon as its accumulation closes:
                # bf16 tmp so the ew multiply runs in 16-bit 2x mode and
                # reads the ew broadcast straight out of the packed input
                if not last:
                    nc.vector.tensor_tensor(
                        tmps[g][:], ps[g][:],
                        b32[:, g:g + 1].broadcast_to([P, B]),
                        mybir.AluOpType.add,
                    )
                    nc.vector.tensor_tensor(
                        zsb[:, g * B:(g + 1) * B], tmps[g][:],
                        inp[:, EWB_C:EWB_C + B],
                        mybir.AluOpType.mult,
                    )
                else:
                    for h in range(2):
                        nc.vector.tensor_tensor(
                            zsb[:, g * B + h * 256:g * B + (h + 1) * 256],
                            tmps[g][:, h * 256:(h + 1) * 256],
                            inp[:, EWB_C + h * 256:EWB_C + (h + 1) * 256],
                            mybir.AluOpType.mult,
                        )
                if g in (3, 6, 7):
                    lo = 0 if g == 3 else (4 * B if g == 6 else 7 * B)
                    hi = (g + 1) * B
                    nc.sync.dma_start(z_d[:, lo:hi], zsb[:, lo:hi])

    return nc


def _get_compiled():
    global _compiled
    if _compiled is None:
        _compiled = _build()
    return _compiled


_prep_cache = None


def _make_in_maps(x, expert_weights, weight, bias):
    global _prep_cache
    import ml_dtypes

    bf = ml_dtypes.bfloat16
    if _prep_cache is None or _prep_cache[0] is not weight:
        wt = np.asarray(weight, dtype=np.float32)
        # wg[p, (g,i,c)] = W[k, g*128+c, i*128+p]
        wgs = [
            wt[k].T.reshape(NI, P, NG, P)
            .transpose(1, 2, 0, 3).reshape(P, NG * NI * P).astype(bf)
            for k in range(NCORES)
        ]
        _prep_cache = (weight, wgs)
    wgs = _prep_cache[1]
    # xT chunks: xt[p, i*512+b] = x[b, i*128+p] — same bytes every core
    xt = (np.asarray(x, dtype=np.float32).T.reshape(NI, P, B)
          .transpose(1, 0, 2).reshape(P, NI * B).astype(bf))
    ew = np.asarray(expert_weights, dtype=np.float32)
    bs = np.asarray(bias, dtype=np.float32)
    in_maps = []
    for k in range(NCORES):
        inp = np.empty((P, IN_COLS), dtype=bf)
        inp[:, XTA_C:XTA_C + 4 * B] = xt[:, :4 * B]
        inp[:, XTB_C:XTB_C + 4 * B] = xt[:, 4 * B:]
        inp[:, WG0_C:WG0_C + NI * P] = wgs[k][:, :NI * P]
        inp[:, WGR_C:IN_COLS] = wgs[k][:, NI * P:]
        inp[:, EWB_C:EWB_C + B] = np.broadcast_to(ew[:, k], (P, B))
        inp[:, BIAS_C:BIAS_C + NG] = bs[k].reshape(NG, P).T
        in_maps.append({"inp": inp})
    return in_maps


def kernel(x, expert_weights, weight, bias, _trace=False):
    from concourse.bass_utils import run_bass_kernel_spmd

    nc = _get_compiled()
    in_maps = _make_in_maps(x, expert_weights, weight, bias)
    res = run_bass_kernel_spmd(
        nc, in_maps, core_ids=list(range(NCORES)), trace=_trace
    )
    y = np.zeros((B, OUT), dtype=np.float32)
    for r in res.results:
        z = np.asarray(r["z"]).astype(np.float32)  # [128, 8*512]
        y += z.reshape(P, NG, B).transpose(1, 0, 2).reshape(OUT, B).T
    if _trace:
        return y, res
    return y
